# revision 8
# baseline (speedup 1.0000x reference)
"""Mixtral block-sparse top-2 MLP with HQQ 4-bit quantized weights, on 8 trn2 cores.

Math (per reference):
    W = (W_q - zero[g, k]) * scale[g, k],  g = out_row // 64
    gate = x @ W1^T ; up = x @ W3^T ; inter = silu(gate) * up ; out = inter @ W2^T

Distribution: shard the ffn dim F across 8 cores (w1/w3 column shards of the
transposed weights, w2 row shards); each core computes a partial out [T, H],
per-h-chunk ReduceScatter sums + scatters token rows, host concatenates.

Device algebra per projection (avoids per-element zero subtraction):
    out[t, n] = sum_k x[t,k]*s[g,k]*Wq[n,k] - zb[g(n), t]
    zb[g, t]  = sum_k (s*z)[g,k] * x[t,k]          (tiny side matmul)
The zb broadcast-subtract is folded into the PSUM accumulation as one extra
matmul with a constant block-diagonal 0/1 selector.
Dequant of Wq (uint8) -> fp16 is one wide DVE multiply per k-tile with the
scale broadcast along the free dim via a 0-step access pattern.

All operands are host-retiled to partition-major [128, ...] blocks so each
logical tensor loads with O(1) large DMAs (DMA issue costs ~0.6us each).
"""

import numpy as np
from contextlib import ExitStack
from dataclasses import dataclass


@dataclass(frozen=True)
class Cfg:
    H: int = 4096      # hidden
    F: int = 14336     # ffn (sharded)
    T: int = 512       # tokens
    NC: int = 8        # cores
    GS: int = 64       # HQQ group size along out rows

    @property
    def FC(self): return self.F // self.NC          # ffn per core
    @property
    def GC(self): return self.FC // self.GS         # w1/w3 groups per core
    @property
    def G2(self): return self.H // self.GS          # w2 groups (H not sharded)
    @property
    def KT(self): return self.H // 128              # k tiles (contraction of w1/w3)
    @property
    def NT(self): return self.FC // 128             # n tiles per core
    @property
    def TT(self): return self.T // 128              # token tiles
    @property
    def HC(self): return self.H // 512              # h chunks of 512 (w2 out)
    @property
    def HCP(self): return self.HC // 2              # h chunk pairs
    @property
    def RS(self): return self.T // self.NC          # rows per core after reduce-scatter
    @property
    def XCH(self): return min(4, self.KT)           # k tiles per x-load chunk
    @property
    def WCH(self): return 2                         # k tiles per weight-stage chunk


CFG = Cfg()


def _tile128(a):
    """[(Nt*128), W] -> [128, Nt*W], partition-major blocks."""
    n, w = a.shape
    assert n % 128 == 0
    return np.ascontiguousarray(
        a.reshape(n // 128, 128, w).transpose(1, 0, 2).reshape(128, -1))


# ---------------------------------------------------------------- host prep

def host_prep(cfg, hidden_states, w1_q, w1_scale, w1_zero,
              w2_q, w2_scale, w2_zero, w3_q, w3_scale, w3_zero):
    """Build per-core input maps (layout/dtype marshaling only)."""
    f16, u8 = np.float16, np.uint8
    NC, FC, GS, GC = cfg.NC, cfg.FC, cfg.GS, cfg.GC

    xT = _tile128(hidden_states.T.astype(f16))                  # [128, KT*T]

    w1T = w1_q.astype(u8).T                                     # [H, F]
    w3T = w3_q.astype(u8).T
    w2T = w2_q.astype(u8).T                                     # [F, H]
    s1T = w1_scale.astype(f16).T                                # [H, F/GS]
    z1T = w1_zero.astype(f16).T
    s3T = w3_scale.astype(f16).T
    z3T = w3_zero.astype(f16).T
    s2T = w2_scale.astype(f16).T                                # [F, H/GS]
    z2T = w2_zero.astype(f16).T

    sel1 = np.zeros((GC, FC), f16)                              # block-diag ones
    for g in range(GC):
        sel1[g, g * GS:(g + 1) * GS] = 1
    sel2 = np.zeros((cfg.G2, cfg.H), f16)
    for g in range(cfg.G2):
        sel2[g, g * GS:(g + 1) * GS] = 1

    def w2_retile(w2c):
        # [FC, H] -> [128, HCP * NT * 1024], h-chunk-pair major
        a = w2c.reshape(cfg.NT, 128, cfg.HCP, 1024)
        return np.ascontiguousarray(
            a.transpose(1, 2, 0, 3).reshape(128, -1))

    maps = []
    for c in range(NC):
        fs = slice(c * FC, (c + 1) * FC)
        gs_ = slice(c * GC, (c + 1) * GC)
        maps.append({
            "xT": xT,
            "w1t": _tile128(np.ascontiguousarray(w1T[:, fs])),
            "w3t": _tile128(np.ascontiguousarray(w3T[:, fs])),
            "w2t": w2_retile(np.ascontiguousarray(w2T[fs, :])),
            "s1t": _tile128(np.ascontiguousarray(s1T[:, gs_])),
            "z1t": _tile128(np.ascontiguousarray(z1T[:, gs_])),
            "s3t": _tile128(np.ascontiguousarray(s3T[:, gs_])),
            "z3t": _tile128(np.ascontiguousarray(z3T[:, gs_])),
            "s2t": _tile128(np.ascontiguousarray(s2T[fs, :])),
            "z2t": _tile128(np.ascontiguousarray(z2T[fs, :])),
            "sel1": sel1,
            "sel2": sel2,
        })
    return maps


# ---------------------------------------------------------------- device body

def emit_body(tc, cfg, io):
    """Emit the per-core program. io: dict name -> DRAM AP."""
    import concourse.mybir as mybir
    nc = tc.nc
    f16, f32, u8 = mybir.dt.float16, mybir.dt.float32, mybir.dt.uint8
    Act = mybir.ActivationFunctionType
    mult = mybir.AluOpType.mult

    KT, NT, TT, HC = cfg.KT, cfg.NT, cfg.TT, cfg.HC
    T, FC, GC, G2, GS = cfg.T, cfg.FC, cfg.GC, cfg.G2, cfg.GS
    XCH, WCH, HCP = cfg.XCH, cfg.WCH, cfg.HCP
    NH1 = (NT + 1) // 2
    half1 = list(range(NH1))
    half2 = list(range(NH1, NT))

    with ExitStack() as ctx:
        # ---- pools that live for the whole kernel
        cp = ctx.enter_context(tc.tile_pool(name="cp", bufs=1))
        silup = ctx.enter_context(tc.tile_pool(name="silup", bufs=1))
        psA = ctx.enter_context(tc.tile_pool(name="psA", bufs=7, space="PSUM"))
        psZ = ctx.enter_context(tc.tile_pool(name="psZ", bufs=1, space="PSUM"))
        dramp = ctx.enter_context(tc.tile_pool(name="dramp", bufs=1, space="DRAM"))
        w2u8p = ctx.enter_context(tc.tile_pool(name="w2u8p", bufs=1))
        # ---- pools released after the gate/up phases (space reused by w2)
        bc = ExitStack()
        xp = bc.enter_context(tc.tile_pool(name="xp", bufs=1))
        cpb = bc.enter_context(tc.tile_pool(name="cpb", bufs=1))
        stagep = bc.enter_context(tc.tile_pool(name="stagep", bufs=2))
        wmp = bc.enter_context(tc.tile_pool(name="wmp", bufs=32))
        sgp = bc.enter_context(tc.tile_pool(name="sgp", bufs=4))

        # ---- scales/zeros: one DMA per tensor, one DVE mult for zs
        def load_sz(sname, zname, ntiles, width, pool):
            sall = pool.tile([128, ntiles * width], f16, name=f"{sname}_all")
            zall = pool.tile([128, ntiles * width], f16, name=f"{zname}_all")
            nc.sync.dma_start(sall[:], io[sname][:])
            nc.sync.dma_start(zall[:], io[zname][:])
            zs = pool.tile([128, ntiles * width], f16, name=f"zs_{sname}")
            nc.vector.tensor_tensor(zs[:], sall[:], zall[:], mult)
            ss = [sall[:, a * width:(a + 1) * width] for a in range(ntiles)]
            pp = [zs[:, a * width:(a + 1) * width] for a in range(ntiles)]
            return ss, pp

        s1_t, zs1_t = load_sz("s1t", "z1t", KT, GC, cpb)
        s3_t, zs3_t = load_sz("s3t", "z3t", KT, GC, cpb)
        s2_t, zs2_t = load_sz("s2t", "z2t", NT, G2, cp)

        sel1_t = cpb.tile([GC, FC], f16)
        nc.sync.dma_start(sel1_t[:], io["sel1"][:])

        # ---- x: XCH k-tiles per chunk, one DMA each (after the small loads)
        x_t = []
        for ch in range(KT // XCH):
            xc = xp.tile([128, XCH * T], f16, name=f"xc{ch}")
            nc.sync.dma_start(xc[:], io["xT"][:, ch * XCH * T:(ch + 1) * XCH * T])
            for a in range(XCH):
                x_t.append(xc[:, a * T:(a + 1) * T])

        # ---- gate/up projection phase (shared emitter)
        def proj_phase(w_name, s_tiles, zs_tiles, evac):
            zb_ps = psZ.tile([GC, T], f32, name="zbps")
            zbn = cpb.tile([GC, T], f16, name=f"zbn_{w_name}")

            # dequant: wm[a] = fp16(u8 * s); staged WCH k-tiles per DMA
            wm = []
            for ch in range(KT // WCH):
                u8t = stagep.tile([128, WCH * FC], u8, name="wstage")
                nc.gpsimd.dma_start(
                    u8t[:], io[w_name][:, ch * WCH * FC:(ch + 1) * WCH * FC])
                for i in range(WCH):
                    a = ch * WCH + i
                    wmt = wmp.tile([128, FC], f16, name="wm")
                    nc.vector.tensor_tensor(
                        wmt[:].rearrange("k (g z) -> k g z", z=GS),
                        u8t[:, i * FC:(i + 1) * FC]
                           .rearrange("k (g z) -> k g z", z=GS),
                        s_tiles[a].unsqueeze(2).broadcast_to([128, GC, GS]),
                        mult)
                    wm.append(wmt)

            # two n-halves; second half consumes k in reverse so wm slots
            # free in the order the next phase's dequants want them
            for js, a_order in ((half1, range(KT)), (half2, range(KT - 1, -1, -1))):
                first_half = js[0] == 0
                ps = [psA.tile([128, T], f32, name="mmps") for _ in js]
                first_a = None
                for a in a_order:
                    if first_a is None:
                        first_a = a
                    for ji, j in enumerate(js):
                        nc.tensor.matmul(ps[ji][:],
                                         wm[a][:, j * 128:(j + 1) * 128],
                                         x_t[a],
                                         start=(a == first_a), stop=False)
                    if first_half:
                        # fold the zero-term side matmul into this sweep
                        nc.tensor.matmul(zb_ps[:], zs_tiles[a], x_t[a],
                                         start=(a == 0), stop=(a == KT - 1))
                if first_half:
                    nc.scalar.activation(zbn[:], zb_ps[:], Act.Copy, scale=-1.0)
                for ji, j in enumerate(js):
                    nc.tensor.matmul(ps[ji][:],
                                     sel1_t[:, j * 128:(j + 1) * 128],
                                     zbn[:], start=False, stop=True)
                    evac(j, ps[ji])

        silu16 = [None] * NT

        def evac_gate(j, ps):
            sg = sgp.tile([128, T], f16, name="sg")
            nc.scalar.activation(sg[:], ps[:], Act.Sigmoid)
            st = silup.tile([128, T], f16, name=f"silu_{j}")
            nc.vector.tensor_tensor(st[:], ps[:], sg[:], mult)   # silu = ps * sig(ps)
            silu16[j] = st

        inter16 = [None] * NT
        zb2_ps_box = []

        def evac_up(j, ps):
            it = silu16[j]               # in-place: inter = up * silu(gate)
            nc.vector.tensor_tensor(it[:], ps[:], it[:], mult)
            inter16[j] = it
            # fold this n-tile into the w2 zero-term as soon as it exists
            if not zb2_ps_box:
                zb2_ps_box.append(psZ.tile([G2, T], f32, name="zbps"))
            nc.tensor.matmul(zb2_ps_box[0][:], zs2_t[j], it[:],
                             start=(j == 0), stop=(j == NT - 1))

        proj_phase("w1t", s1_t, zs1_t, evac_gate)
        proj_phase("w3t", s3_t, zs3_t, evac_up)

        # prefetch the first w2 u8 block while gate/up still run
        blk = NT * 1024
        w2u8 = [w2u8p.tile([128, blk], u8, name="w2stage")]
        nc.gpsimd.dma_start(w2u8[0][:], io["w2t"][:, 0:blk])

        # ---- release gate/up pools so the w2 phase reuses their SBUF
        bc.close()

        # ---- w2 phase: out[t, h] = sum_n inter[n, t] * wm2[n, h] - zb2[g(h), t]
        zb2n = cp.tile([G2, T], f16)
        nc.scalar.activation(zb2n[:], zb2_ps_box[0][:], Act.Copy, scale=-1.0)

        with tc.tile_pool(name="sel2p", bufs=1) as sel2p, \
             tc.tile_pool(name="w2sp", bufs=2 * NT) as w2sp, \
             tc.tile_pool(name="outp", bufs=3) as outp:
            sel2_t = sel2p.tile([G2, cfg.H], f16)
            nc.sync.dma_start(sel2_t[:], io["sel2"][:])

            GPC2 = 1024 // GS            # groups per 1024-wide h pair-chunk
            part_hc = []                 # per-half-h-chunk partials [T, 256] in DRAM
            for hc in range(HC):
                part_hc.append([dramp.tile([T, 256], f16, name=f"part{hc}_{s}")
                                for s in range(2)])

            for hp in range(HCP):
                u8b = w2u8[hp]
                if hp + 1 < HCP:         # prefetch next block ahead of RS triggers
                    nxt = w2u8p.tile([128, blk], u8, name="w2stage")
                    nc.gpsimd.dma_start(
                        nxt[:], io["w2t"][:, (hp + 1) * blk:(hp + 2) * blk])
                    w2u8.append(nxt)
                strips = []
                for j in range(NT):
                    w2s = w2sp.tile([128, 1024], f16, name="w2s")
                    nc.vector.tensor_tensor(
                        w2s[:].rearrange("k (g z) -> k g z", z=GS),
                        u8b[:, j * 1024:(j + 1) * 1024]
                           .rearrange("k (g z) -> k g z", z=GS),
                        s2_t[j][:, hp * GPC2:(hp + 1) * GPC2]
                            .unsqueeze(2).broadcast_to([128, GPC2, GS]),
                        mult)
                    strips.append(w2s)
                for hh in range(2):
                    hc = hp * 2 + hh
                    outsb = outp.tile([128, TT * 512], f16, name="outevac")
                    for tt in range(TT):
                        ps = psA.tile([128, 512], f32, name="mmps")
                        for j in range(NT):
                            nc.tensor.matmul(
                                ps[:],
                                inter16[j][:, tt * 128:(tt + 1) * 128],
                                strips[j][:, hh * 512:(hh + 1) * 512],
                                start=(j == 0), stop=False)
                        nc.tensor.matmul(
                            ps[:],
                            zb2n[:, tt * 128:(tt + 1) * 128],
                            sel2_t[:, hc * 512:(hc + 1) * 512],
                            start=False, stop=True)
                        nc.scalar.activation(
                            outsb[:, tt * 512:(tt + 1) * 512], ps[:], Act.Copy)
                    for s in range(2):
                        nc.sync.dma_start(
                            part_hc[hc][s][:].rearrange("(b p) h -> p b h", p=128),
                            outsb[:].rearrange("p (b h) -> p b h", h=512)
                                 [:, :, s * 256:(s + 1) * 256])
                        # reduce-scatter while later chunks compute
                        rs_out = dramp.tile([cfg.RS, 256], f16, name=f"rs{hc}_{s}")
                        nc.gpsimd.collective_compute(
                            "ReduceScatter", mybir.AluOpType.add,
                            replica_groups=[list(range(cfg.NC))],
                            ins=[part_hc[hc][s].opt()], outs=[rs_out.opt()])
                        of16 = outp.tile([cfg.RS, 256], f16, name="of16")
                        nc.scalar.dma_start(of16[:], rs_out[:])
                        of32 = outp.tile([cfg.RS, 256], f32, name="of32")
                        nc.scalar.activation(of32[:], of16[:], Act.Copy)
                        nc.scalar.dma_start(
                            io["out"][:, hc * 512 + s * 256:
                                      hc * 512 + (s + 1) * 256], of32[:])


# ---------------------------------------------------------------- build + run

def build_program(cfg):
    import concourse.bacc as bacc
    import concourse.mybir as mybir
    from concourse import tile

    f16, f32, u8 = mybir.dt.float16, mybir.dt.float32, mybir.dt.uint8
    nc = bacc.Bacc("TRN2", target_bir_lowering=False, debug=False,
                   num_devices=cfg.NC)
    KT, NT, GC, G2 = cfg.KT, cfg.NT, cfg.GC, cfg.G2

    def din(name, shape, dt):
        return nc.dram_tensor(name, shape, dt, kind="ExternalInput").ap()

    io = {
        "xT": din("xT", [128, KT * cfg.T], f16),
        "w1t": din("w1t", [128, KT * cfg.FC], u8),
        "w3t": din("w3t", [128, KT * cfg.FC], u8),
        "w2t": din("w2t", [128, cfg.HCP * NT * 1024], u8),
        "s1t": din("s1t", [128, KT * GC], f16),
        "z1t": din("z1t", [128, KT * GC], f16),
        "s3t": din("s3t", [128, KT * GC], f16),
        "z3t": din("z3t", [128, KT * GC], f16),
        "s2t": din("s2t", [128, NT * G2], f16),
        "z2t": din("z2t", [128, NT * G2], f16),
        "sel1": din("sel1", [GC, cfg.FC], f16),
        "sel2": din("sel2", [G2, cfg.H], f16),
        "out": nc.dram_tensor("out", [cfg.RS, cfg.H], f32,
                              kind="ExternalOutput").ap(),
    }
    with tile.TileContext(nc) as tc:
        emit_body(tc, cfg, io)
    nc.compile()
    return nc


_PROGRAM = None


def kernel(**inputs) -> np.ndarray:
    from concourse.bass_utils import run_bass_kernel_spmd

    global _PROGRAM
    cfg = CFG
    if _PROGRAM is None:
        _PROGRAM = build_program(cfg)
    in_maps = host_prep(cfg, **inputs)
    res = run_bass_kernel_spmd(_PROGRAM, in_maps, list(range(cfg.NC)))
    return np.concatenate([res.results[c]["out"] for c in range(cfg.NC)], axis=0)


# revision 9
# speedup vs baseline: 1.1477x; 1.1477x over previous
"""Mixtral block-sparse top-2 MLP with HQQ 4-bit quantized weights, on 8 trn2 cores.

Math (per reference):
    W = (W_q - zero[g, k]) * scale[g, k],  g = out_row // 64
    gate = x @ W1^T ; up = x @ W3^T ; inter = silu(gate) * up ; out = inter @ W2^T

Distribution: shard the ffn dim F across 8 cores (w1/w3 column shards of the
transposed weights, w2 row shards); each core computes a partial out [T, H],
per-h-chunk ReduceScatter sums + scatters token rows, host concatenates.

Device algebra per projection (avoids per-element zero subtraction):
    out[t, n] = sum_k x[t,k]*s[g,k]*Wq[n,k] - zb[g(n), t]
    zb[g, t]  = sum_k (s*z)[g,k] * x[t,k]          (tiny side matmul)
The zb broadcast-subtract is folded into the PSUM accumulation as one extra
matmul with a constant block-diagonal 0/1 selector.
Dequant of Wq (uint8) -> fp16 is one wide DVE multiply per k-tile with the
scale broadcast along the free dim via a 0-step access pattern.

All operands are host-retiled to partition-major [128, ...] blocks so each
logical tensor loads with O(1) large DMAs (DMA issue costs ~0.6us each).
"""

import numpy as np
from contextlib import ExitStack
from dataclasses import dataclass


@dataclass(frozen=True)
class Cfg:
    H: int = 4096      # hidden
    F: int = 14336     # ffn (sharded)
    T: int = 512       # tokens
    NC: int = 8        # cores
    GS: int = 64       # HQQ group size along out rows

    @property
    def FC(self): return self.F // self.NC          # ffn per core
    @property
    def GC(self): return self.FC // self.GS         # w1/w3 groups per core
    @property
    def G2(self): return self.H // self.GS          # w2 groups (H not sharded)
    @property
    def KT(self): return self.H // 128              # k tiles (contraction of w1/w3)
    @property
    def NT(self): return self.FC // 128             # n tiles per core
    @property
    def TT(self): return self.T // 128              # token tiles
    @property
    def HC(self): return self.H // 512              # h chunks of 512 (w2 out)
    @property
    def HCP(self): return self.HC // 2              # h chunk pairs
    @property
    def RS(self): return self.T // self.NC          # rows per core after reduce-scatter
    @property
    def XCH(self): return min(4, self.KT)           # k tiles per x-load chunk
    @property
    def WCH(self): return 2                         # k tiles per weight-stage chunk


CFG = Cfg()


def _tile128(a):
    """[(Nt*128), W] -> [128, Nt*W], partition-major blocks."""
    n, w = a.shape
    assert n % 128 == 0
    return np.ascontiguousarray(
        a.reshape(n // 128, 128, w).transpose(1, 0, 2).reshape(128, -1))


# ---------------------------------------------------------------- host prep

def host_prep(cfg, hidden_states, w1_q, w1_scale, w1_zero,
              w2_q, w2_scale, w2_zero, w3_q, w3_scale, w3_zero):
    """Build per-core input maps (layout/dtype marshaling only)."""
    f16, u8 = np.float16, np.uint8
    NC, FC, GS, GC = cfg.NC, cfg.FC, cfg.GS, cfg.GC

    xT = _tile128(hidden_states.T.astype(f16))                  # [128, KT*T]

    w1T = w1_q.astype(u8).T                                     # [H, F]
    w3T = w3_q.astype(u8).T
    w2T = w2_q.astype(u8).T                                     # [F, H]
    s1T = w1_scale.astype(f16).T                                # [H, F/GS]
    z1T = w1_zero.astype(f16).T
    s3T = w3_scale.astype(f16).T
    z3T = w3_zero.astype(f16).T
    s2T = w2_scale.astype(f16).T                                # [F, H/GS]
    z2T = w2_zero.astype(f16).T

    sel1 = np.zeros((GC, FC), f16)                              # block-diag ones
    for g in range(GC):
        sel1[g, g * GS:(g + 1) * GS] = 1
    sel2 = np.zeros((cfg.G2, cfg.H), f16)
    for g in range(cfg.G2):
        sel2[g, g * GS:(g + 1) * GS] = 1

    def w2_retile(w2c):
        # [FC, H] -> [128, HCP * NT * 1024], h-chunk-pair major
        a = w2c.reshape(cfg.NT, 128, cfg.HCP, 1024)
        return np.ascontiguousarray(
            a.transpose(1, 2, 0, 3).reshape(128, -1))

    maps = []
    for c in range(NC):
        fs = slice(c * FC, (c + 1) * FC)
        gs_ = slice(c * GC, (c + 1) * GC)
        maps.append({
            "xT": xT,
            "w1t": _tile128(np.ascontiguousarray(w1T[:, fs])),
            "w3t": _tile128(np.ascontiguousarray(w3T[:, fs])),
            "w2t": w2_retile(np.ascontiguousarray(w2T[fs, :])),
            "s1t": _tile128(np.ascontiguousarray(s1T[:, gs_])),
            "z1t": _tile128(np.ascontiguousarray(z1T[:, gs_])),
            "s3t": _tile128(np.ascontiguousarray(s3T[:, gs_])),
            "z3t": _tile128(np.ascontiguousarray(z3T[:, gs_])),
            "s2t": _tile128(np.ascontiguousarray(s2T[fs, :])),
            "z2t": _tile128(np.ascontiguousarray(z2T[fs, :])),
            "sel1": sel1,
            "sel2": sel2,
        })
    return maps


# ---------------------------------------------------------------- device body

def emit_body(tc, cfg, io):
    """Emit the per-core program. io: dict name -> DRAM AP."""
    import concourse.mybir as mybir
    nc = tc.nc
    f16, f32, u8 = mybir.dt.float16, mybir.dt.float32, mybir.dt.uint8
    Act = mybir.ActivationFunctionType
    mult = mybir.AluOpType.mult

    KT, NT, TT, HC = cfg.KT, cfg.NT, cfg.TT, cfg.HC
    T, FC, GC, G2, GS = cfg.T, cfg.FC, cfg.GC, cfg.G2, cfg.GS
    XCH, WCH, HCP = cfg.XCH, cfg.WCH, cfg.HCP
    NH1 = (NT + 1) // 2
    half1 = list(range(NH1))
    half2 = list(range(NH1, NT))

    with ExitStack() as ctx:
        # ---- pools that live for the whole kernel
        cp = ctx.enter_context(tc.tile_pool(name="cp", bufs=1))
        silup = ctx.enter_context(tc.tile_pool(name="silup", bufs=1))
        psA = ctx.enter_context(tc.tile_pool(name="psA", bufs=7, space="PSUM"))
        psZ = ctx.enter_context(tc.tile_pool(name="psZ", bufs=1, space="PSUM"))
        dramp = ctx.enter_context(tc.tile_pool(name="dramp", bufs=1, space="DRAM"))
        w2u8p = ctx.enter_context(tc.tile_pool(name="w2u8p", bufs=1))
        # ---- pools released after the gate/up phases (space reused by w2)
        bc = ExitStack()
        xp = bc.enter_context(tc.tile_pool(name="xp", bufs=1))
        cpb = bc.enter_context(tc.tile_pool(name="cpb", bufs=1))
        stagep = bc.enter_context(tc.tile_pool(name="stagep", bufs=2))
        wmp = bc.enter_context(tc.tile_pool(name="wmp", bufs=32))
        sgp = bc.enter_context(tc.tile_pool(name="sgp", bufs=4))

        # ---- scales/zeros: one DMA per tensor, one DVE mult for zs
        def load_sz(sname, zname, ntiles, width, pool):
            sall = pool.tile([128, ntiles * width], f16, name=f"{sname}_all")
            zall = pool.tile([128, ntiles * width], f16, name=f"{zname}_all")
            nc.sync.dma_start(sall[:], io[sname][:])
            nc.sync.dma_start(zall[:], io[zname][:])
            zs = pool.tile([128, ntiles * width], f16, name=f"zs_{sname}")
            nc.vector.tensor_tensor(zs[:], sall[:], zall[:], mult)
            ss = [sall[:, a * width:(a + 1) * width] for a in range(ntiles)]
            pp = [zs[:, a * width:(a + 1) * width] for a in range(ntiles)]
            return ss, pp

        s1_t, zs1_t = load_sz("s1t", "z1t", KT, GC, cpb)

        sel1_t = cpb.tile([GC, FC], f16)
        nc.sync.dma_start(sel1_t[:], io["sel1"][:])

        # ---- x: XCH k-tiles per chunk, one DMA each
        x_t = []
        for ch in range(KT // XCH):
            xc = xp.tile([128, XCH * T], f16, name=f"xc{ch}")
            nc.sync.dma_start(xc[:], io["xT"][:, ch * XCH * T:(ch + 1) * XCH * T])
            for a in range(XCH):
                x_t.append(xc[:, a * T:(a + 1) * T])

        s3_t, zs3_t = load_sz("s3t", "z3t", KT, GC, cpb)
        s2_t, zs2_t = load_sz("s2t", "z2t", NT, G2, cp)

        # ---- gate/up projection phase (shared emitter)
        def proj_phase(w_name, s_tiles, zs_tiles, evac):
            zb_ps = psZ.tile([GC, T], f32, name="zbps")
            zbn = cpb.tile([GC, T], f16, name=f"zbn_{w_name}")

            # dequant: wm[a] = fp16(u8 * s); staged WCH k-tiles per DMA
            wm = []
            for ch in range(KT // WCH):
                u8t = stagep.tile([128, WCH * FC], u8, name="wstage")
                nc.gpsimd.dma_start(
                    u8t[:], io[w_name][:, ch * WCH * FC:(ch + 1) * WCH * FC])
                for i in range(WCH):
                    a = ch * WCH + i
                    wmt = wmp.tile([128, FC], f16, name="wm")
                    nc.vector.tensor_tensor(
                        wmt[:].rearrange("k (g z) -> k g z", z=GS),
                        u8t[:, i * FC:(i + 1) * FC]
                           .rearrange("k (g z) -> k g z", z=GS),
                        s_tiles[a].unsqueeze(2).broadcast_to([128, GC, GS]),
                        mult)
                    wm.append(wmt)

            # two n-halves; second half consumes k in reverse so wm slots
            # free in the order the next phase's dequants want them
            for js, a_order in ((half1, range(KT)), (half2, range(KT - 1, -1, -1))):
                first_half = js[0] == 0
                ps = [psA.tile([128, T], f32, name="mmps") for _ in js]
                first_a = None
                for a in a_order:
                    if first_a is None:
                        first_a = a
                    for ji, j in enumerate(js):
                        nc.tensor.matmul(ps[ji][:],
                                         wm[a][:, j * 128:(j + 1) * 128],
                                         x_t[a],
                                         start=(a == first_a), stop=False)
                    if first_half:
                        # fold the zero-term side matmul into this sweep
                        nc.tensor.matmul(zb_ps[:], zs_tiles[a], x_t[a],
                                         start=(a == 0), stop=(a == KT - 1))
                if first_half:
                    nc.scalar.activation(zbn[:], zb_ps[:], Act.Copy, scale=-1.0)
                for ji, j in enumerate(js):
                    nc.tensor.matmul(ps[ji][:],
                                     sel1_t[:, j * 128:(j + 1) * 128],
                                     zbn[:], start=False, stop=True)
                    evac(j, ps[ji])

        silu16 = [None] * NT

        def evac_gate(j, ps):
            sg = sgp.tile([128, T], f16, name="sg")
            nc.scalar.activation(sg[:], ps[:], Act.Sigmoid)
            st = silup.tile([128, T], f16, name=f"silu_{j}")
            nc.vector.tensor_tensor(st[:], ps[:], sg[:], mult)   # silu = ps * sig(ps)
            silu16[j] = st

        inter16 = [None] * NT
        zb2_ps_box = []

        def evac_up(j, ps):
            it = silu16[j]               # in-place: inter = up * silu(gate)
            nc.vector.tensor_tensor(it[:], ps[:], it[:], mult)
            inter16[j] = it
            # fold this n-tile into the w2 zero-term as soon as it exists
            if not zb2_ps_box:
                zb2_ps_box.append(psZ.tile([G2, T], f32, name="zbps"))
            nc.tensor.matmul(zb2_ps_box[0][:], zs2_t[j], it[:],
                             start=(j == 0), stop=(j == NT - 1))

        proj_phase("w1t", s1_t, zs1_t, evac_gate)
        proj_phase("w3t", s3_t, zs3_t, evac_up)

        # prefetch the first w2 u8 block while gate/up still run
        blk = NT * 1024
        w2u8 = [w2u8p.tile([128, blk], u8, name="w2stage")]
        nc.gpsimd.dma_start(w2u8[0][:], io["w2t"][:, 0:blk])

        # ---- release gate/up pools so the w2 phase reuses their SBUF
        bc.close()

        # ---- w2 phase: out[t, h] = sum_n inter[n, t] * wm2[n, h] - zb2[g(h), t]
        zb2n = cp.tile([G2, T], f16)
        nc.scalar.activation(zb2n[:], zb2_ps_box[0][:], Act.Copy, scale=-1.0)

        with tc.tile_pool(name="sel2p", bufs=1) as sel2p, \
             tc.tile_pool(name="w2sp", bufs=2 * NT) as w2sp, \
             tc.tile_pool(name="outp", bufs=3) as outp:
            sel2_t = sel2p.tile([G2, cfg.H], f16)
            nc.sync.dma_start(sel2_t[:], io["sel2"][:])

            GPC2 = 1024 // GS            # groups per 1024-wide h pair-chunk
            part_hp = []                 # per-h-chunk-pair partials [T, 1024] in DRAM
            for hp in range(HCP):
                part_hp.append(dramp.tile([T, 1024], f16, name=f"part{hp}"))

            for hp in range(HCP):
                u8b = w2u8[hp]
                if hp + 1 < HCP:         # prefetch next block ahead of RS triggers
                    nxt = w2u8p.tile([128, blk], u8, name="w2stage")
                    nc.gpsimd.dma_start(
                        nxt[:], io["w2t"][:, (hp + 1) * blk:(hp + 2) * blk])
                    w2u8.append(nxt)
                strips = []
                for j in range(NT):
                    w2s = w2sp.tile([128, 1024], f16, name="w2s")
                    nc.vector.tensor_tensor(
                        w2s[:].rearrange("k (g z) -> k g z", z=GS),
                        u8b[:, j * 1024:(j + 1) * 1024]
                           .rearrange("k (g z) -> k g z", z=GS),
                        s2_t[j][:, hp * GPC2:(hp + 1) * GPC2]
                            .unsqueeze(2).broadcast_to([128, GPC2, GS]),
                        mult)
                    strips.append(w2s)
                for hh in range(2):
                    hc = hp * 2 + hh
                    outsb = outp.tile([128, TT * 512], f16, name="outevac")
                    for tt in range(TT):
                        ps = psA.tile([128, 512], f32, name="mmps")
                        for j in range(NT):
                            nc.tensor.matmul(
                                ps[:],
                                inter16[j][:, tt * 128:(tt + 1) * 128],
                                strips[j][:, hh * 512:(hh + 1) * 512],
                                start=(j == 0), stop=False)
                        nc.tensor.matmul(
                            ps[:],
                            zb2n[:, tt * 128:(tt + 1) * 128],
                            sel2_t[:, hc * 512:(hc + 1) * 512],
                            start=False, stop=True)
                        nc.scalar.activation(
                            outsb[:, tt * 512:(tt + 1) * 512], ps[:], Act.Copy)
                    nc.sync.dma_start(
                        part_hp[hp][:].rearrange("(b p) h -> p b h", p=128)
                            [:, :, hh * 512:(hh + 1) * 512],
                        outsb[:].rearrange("p (b h) -> p b h", h=512))
                # reduce-scatter this pair while later pairs compute
                rs_out = dramp.tile([cfg.RS, 1024], f16, name=f"rs{hp}")
                nc.gpsimd.collective_compute(
                    "ReduceScatter", mybir.AluOpType.add,
                    replica_groups=[list(range(cfg.NC))],
                    ins=[part_hp[hp].opt()], outs=[rs_out.opt()])
                of16 = outp.tile([cfg.RS, 1024], f16, name="of16")
                nc.scalar.dma_start(of16[:], rs_out[:])
                of32 = outp.tile([cfg.RS, 1024], f32, name="of32")
                nc.scalar.activation(of32[:], of16[:], Act.Copy)
                nc.scalar.dma_start(
                    io["out"][:, hp * 1024:(hp + 1) * 1024], of32[:])


# ---------------------------------------------------------------- build + run

def build_program(cfg):
    import concourse.bacc as bacc
    import concourse.mybir as mybir
    from concourse import tile

    f16, f32, u8 = mybir.dt.float16, mybir.dt.float32, mybir.dt.uint8
    nc = bacc.Bacc("TRN2", target_bir_lowering=False, debug=False,
                   num_devices=cfg.NC)
    KT, NT, GC, G2 = cfg.KT, cfg.NT, cfg.GC, cfg.G2

    def din(name, shape, dt):
        return nc.dram_tensor(name, shape, dt, kind="ExternalInput").ap()

    io = {
        "xT": din("xT", [128, KT * cfg.T], f16),
        "w1t": din("w1t", [128, KT * cfg.FC], u8),
        "w3t": din("w3t", [128, KT * cfg.FC], u8),
        "w2t": din("w2t", [128, cfg.HCP * NT * 1024], u8),
        "s1t": din("s1t", [128, KT * GC], f16),
        "z1t": din("z1t", [128, KT * GC], f16),
        "s3t": din("s3t", [128, KT * GC], f16),
        "z3t": din("z3t", [128, KT * GC], f16),
        "s2t": din("s2t", [128, NT * G2], f16),
        "z2t": din("z2t", [128, NT * G2], f16),
        "sel1": din("sel1", [GC, cfg.FC], f16),
        "sel2": din("sel2", [G2, cfg.H], f16),
        "out": nc.dram_tensor("out", [cfg.RS, cfg.H], f32,
                              kind="ExternalOutput").ap(),
    }
    with tile.TileContext(nc) as tc:
        emit_body(tc, cfg, io)
    nc.compile()
    return nc


_PROGRAM = None


def kernel(**inputs) -> np.ndarray:
    from concourse.bass_utils import run_bass_kernel_spmd

    global _PROGRAM
    cfg = CFG
    if _PROGRAM is None:
        _PROGRAM = build_program(cfg)
    in_maps = host_prep(cfg, **inputs)
    res = run_bass_kernel_spmd(_PROGRAM, in_maps, list(range(cfg.NC)))
    return np.concatenate([res.results[c]["out"] for c in range(cfg.NC)], axis=0)
